# revision 9
# baseline (speedup 1.0000x reference)
"""Masked Hillis-Steele scan kernel for Trainium2 (8 NeuronCores, SPMD).

Problem: B=131072 rows, L=512. For each row:
    y = where(mask, x, 0)
    for s in [1,2,4,...,512]:  # s=512 step is a no-op (shift of full row)
        y[i] += y[i-s]  if mask[i] and mask[i-s]

Key algebraic fact: unmasked positions of y stay 0 forever, so
    mask[i-s]*y[i-s] == y[i-s]  and each step is  y += mask * shift_s(y).

Design (flat/DVE): rows on partitions, L on the free dim, fp16 on-chip.
The shift is a free-dim AP offset (no PE/PSUM involved). The per-step
mul/add run as plain TensorTensor ops, which hit the DVE 2x_1p perf
mode for packed fp16 (~0.52 ns per free element). x is pre-masked on
the host (y0 = mask * x), killing the init multiply. Mask is DMA'd as
uint8 (1/4 the bytes) and cast to fp16 on the otherwise-idle Act
engine. A slice of each tile's row-groups runs on GpSimd (Pool) in
parallel with the DVE to add ~15% throughput.

Sharding: pure data parallel over B across the 8 cores.
"""

import os
import sys

import numpy as np

sys.path.insert(0, "/opt/trn_rl_repo")

B = 131072
L = 512
N_CORES = 8
BP = B // N_CORES  # rows per core = 16384

G = 16  # row-groups per tile: tile = [128 partitions, G groups, L]
ROWS_PER_TILE = 128 * G  # 2048
N_TILES = BP // ROWS_PER_TILE  # 8
G_GP = 3  # row-groups per tile handled by GpSimd (rest on DVE)

SHIFTS = [1, 2, 4, 8, 16, 32, 64, 128, 256]

_last_results = None  # stash for test harness introspection


def _legalize_waits(nc, cap=1):
    """Walrus's TRN2 instruction encodings only have room for a small number
    of sync-wait commands (1 for DMA/3D-AP tensor ops); Tile freely attaches
    more. Hoist surplus waits into standalone event-semaphore (wait-only)
    instructions inserted just before the over-subscribed instruction on the
    same engine queue."""
    import concourse.mybir as mybir

    n_new = 0
    for f in nc.m.functions:
        for b in f.blocks:
            new_list = []
            for ins in b.instructions:
                si = ins.sync_info
                if si is not None and len(si.on_wait) > cap:
                    waits = list(si.on_wait)
                    extra, keep = waits[:-cap], waits[-cap:]
                    for w in extra:
                        ev = mybir.InstEventSemaphore(
                            name=f"waitsplit_{n_new}", ins=[], outs=[]
                        )
                        ev.engine = ins.engine
                        ev.sync_info = mybir.SyncInfo(on_wait=[w], on_update=[])
                        new_list.append(ev)
                        n_new += 1
                    ins.sync_info = mybir.SyncInfo(
                        on_wait=keep, on_update=list(si.on_update)
                    )
                new_list.append(ins)
            b.instructions[:] = new_list
    return n_new


def _build_flat16_program(reps=1, legalize=True):
    """Flat layout [128, G, L] fp16; all compute on DVE via
    scalar_tensor_tensor (4x perf mode); mask u8->fp16 cast on Act.

    reps>1 repeats the whole body (same I/O) for slope-based HW timing."""
    from contextlib import ExitStack

    import concourse.bass as bass
    import concourse.mybir as mybir
    import concourse.tile as tile

    f16 = mybir.dt.float16
    u8 = mybir.dt.uint8
    MUL = mybir.AluOpType.mult
    ADD = mybir.AluOpType.add

    nc = bass.Bass(target_bir_lowering=False, debug=False)
    x_ext = nc.declare_dram_parameter("x", [BP, L], f16, isOutput=False)
    m_ext = nc.declare_dram_parameter("m", [BP, L], u8, isOutput=False)
    y_ext = nc.declare_dram_parameter("y", [BP, L], f16, isOutput=True)

    with tile.TileContext(nc) as tc, ExitStack() as ctx:
        xp = ctx.enter_context(tc.tile_pool(name="xp", bufs=2))
        m8p = ctx.enter_context(tc.tile_pool(name="m8p", bufs=2))
        mp = ctx.enter_context(tc.tile_pool(name="mp", bufs=2))
        tp = ctx.enter_context(tc.tile_pool(name="tp", bufs=2))

        def body(_iv=None):
            for r in range(N_TILES):
                rows = slice(r * ROWS_PER_TILE, (r + 1) * ROWS_PER_TILE)
                xt = xp.tile([128, G, L], f16)
                m8 = m8p.tile([128, G, L], u8)
                mt = mp.tile([128, G, L], f16)
                tt = tp.tile([128, G, L], f16)

                # DRAM row g*128+p -> tile[p, g, :]
                nc.sync.dma_start(
                    xt[:], x_ext[rows, :].rearrange("(g p) l -> p g l", p=128)
                )
                nc.sync.dma_start(
                    m8[:], m_ext[rows, :].rearrange("(g p) l -> p g l", p=128)
                )

                # Act engine: cast mask u8 -> fp16 (keeps DVE free)
                nc.scalar.copy(mt[:], m8[:])

                # x arrives pre-masked from the host (y0 = mask * x), so
                # xt IS y0; run the 9 steps in place on xt.
                gd = G - G_GP  # DVE handles groups [0, gd), GpSimd [gd, G)
                for s in SHIFTS:
                    # t[i] = y[i-s] * m[i]  for i in [s, L);  y[i] += t[i]
                    nc.vector.tensor_mul(
                        tt[:, :gd, s:], xt[:, :gd, : L - s], mt[:, :gd, s:]
                    )
                    nc.vector.tensor_add(
                        xt[:, :gd, s:], xt[:, :gd, s:], tt[:, :gd, s:]
                    )
                    if G_GP:
                        nc.gpsimd.tensor_mul(
                            tt[:, gd:, s:], xt[:, gd:, : L - s], mt[:, gd:, s:]
                        )
                        nc.gpsimd.tensor_add(
                            xt[:, gd:, s:], xt[:, gd:, s:], tt[:, gd:, s:]
                        )

                nc.sync.dma_start(
                    y_ext[rows, :].rearrange("(g p) l -> p g l", p=128), xt[:]
                )

        if reps == 1:
            body()
        else:
            with tc.For_i(0, reps, 1) as iv:
                body(iv)

    if legalize:
        _legalize_waits(nc)
    return nc


_cached = {}


def kernel(x, mask):
    global _last_results
    from concourse.bass_utils import run_bass_kernel_spmd

    x = np.asarray(x)
    m = np.asarray(mask)
    assert x.shape == (B, L) and m.shape == (B, L)
    # Host pre-masking: y0 = where(mask, x, 0) — saves the on-chip init mul
    x16 = np.where(m, x, np.float32(0.0)).astype(np.float16)
    m8 = m.astype(np.uint8)

    if "flat16" not in _cached:
        _cached["flat16"] = _build_flat16_program()
    nc = _cached["flat16"]

    core_ids = list(range(N_CORES))
    in_maps = [
        {
            "x": x16[i * BP : (i + 1) * BP],
            "m": m8[i * BP : (i + 1) * BP],
        }
        for i in core_ids
    ]

    res = run_bass_kernel_spmd(nc, in_maps, core_ids)
    _last_results = res

    out = np.empty((B, L), dtype=np.float32)
    for i in core_ids:
        out[i * BP : (i + 1) * BP] = res.results[i]["y"].astype(np.float32)
    return out


# revision 10
# speedup vs baseline: 1.3872x; 1.3872x over previous
"""Masked Hillis-Steele scan kernel for Trainium2 (8 NeuronCores, SPMD).

Problem: B=131072 rows, L=512. For each row:
    y = where(mask, x, 0)
    for s in [1,2,4,...,512]:  # s=512 step is a no-op (shift of full row)
        y[i] += y[i-s]  if mask[i] and mask[i-s]

Key algebraic fact: unmasked positions of y stay 0 forever, so
    mask[i-s]*y[i-s] == y[i-s]  and each step is  y += mask * shift_s(y).

Design (flat/DVE): rows on partitions, L on the free dim, fp16 on-chip.
The shift is a free-dim AP offset (no PE/PSUM involved). The per-step
mul/add run as plain TensorTensor ops, which hit the DVE 2x_1p perf
mode for packed fp16 (~0.52 ns per free element). x is pre-masked on
the host (y0 = mask * x), killing the init multiply. Mask is DMA'd as
uint8 (1/4 the bytes) and cast to fp16 on the otherwise-idle Act
engine. A slice of each tile's row-groups runs on GpSimd (Pool) in
parallel with the DVE to add ~15% throughput.

Sharding: pure data parallel over B across the 8 cores.
"""

import os
import sys

import numpy as np

sys.path.insert(0, "/opt/trn_rl_repo")

B = 131072
L = 512
N_CORES = 8
BP = B // N_CORES  # rows per core = 16384

G = 16  # row-groups per tile: tile = [128 partitions, G groups, L]
ROWS_PER_TILE = 128 * G  # 2048
N_TILES = BP // ROWS_PER_TILE  # 8
G_GP = 0  # row-groups per tile handled by GpSimd (rest on DVE);
# measured: GpSimd TensorTensor runs ~4.2 ns/elem (8x slower than DVE fp16)
# and sharing tiles with the DVE lock-steps both engines -> disabled

SHIFTS = [1, 2, 4, 8, 16, 32, 64, 128, 256]

_last_results = None  # stash for test harness introspection


def _legalize_waits(nc, cap=1):
    """Walrus's TRN2 instruction encodings only have room for a small number
    of sync-wait commands (1 for DMA/3D-AP tensor ops); Tile freely attaches
    more. Hoist surplus waits into standalone event-semaphore (wait-only)
    instructions inserted just before the over-subscribed instruction on the
    same engine queue."""
    import concourse.mybir as mybir

    n_new = 0
    for f in nc.m.functions:
        for b in f.blocks:
            new_list = []
            for ins in b.instructions:
                si = ins.sync_info
                if si is not None and len(si.on_wait) > cap:
                    waits = list(si.on_wait)
                    extra, keep = waits[:-cap], waits[-cap:]
                    for w in extra:
                        ev = mybir.InstEventSemaphore(
                            name=f"waitsplit_{n_new}", ins=[], outs=[]
                        )
                        ev.engine = ins.engine
                        ev.sync_info = mybir.SyncInfo(on_wait=[w], on_update=[])
                        new_list.append(ev)
                        n_new += 1
                    ins.sync_info = mybir.SyncInfo(
                        on_wait=keep, on_update=list(si.on_update)
                    )
                new_list.append(ins)
            b.instructions[:] = new_list
    return n_new


def _build_flat16_program(reps=1, legalize=True):
    """Flat layout [128, G, L] fp16; all compute on DVE via
    scalar_tensor_tensor (4x perf mode); mask u8->fp16 cast on Act.

    reps>1 repeats the whole body (same I/O) for slope-based HW timing."""
    from contextlib import ExitStack

    import concourse.bass as bass
    import concourse.mybir as mybir
    import concourse.tile as tile

    f16 = mybir.dt.float16
    u8 = mybir.dt.uint8
    MUL = mybir.AluOpType.mult
    ADD = mybir.AluOpType.add

    nc = bass.Bass(target_bir_lowering=False, debug=False)
    x_ext = nc.declare_dram_parameter("x", [BP, L], f16, isOutput=False)
    m_ext = nc.declare_dram_parameter("m", [BP, L], u8, isOutput=False)
    y_ext = nc.declare_dram_parameter("y", [BP, L], f16, isOutput=True)

    with tile.TileContext(nc) as tc, ExitStack() as ctx:
        xp = ctx.enter_context(tc.tile_pool(name="xp", bufs=2))
        m8p = ctx.enter_context(tc.tile_pool(name="m8p", bufs=2))
        mp = ctx.enter_context(tc.tile_pool(name="mp", bufs=2))
        tp = ctx.enter_context(tc.tile_pool(name="tp", bufs=2))

        def body(_iv=None):
            for r in range(N_TILES):
                rows = slice(r * ROWS_PER_TILE, (r + 1) * ROWS_PER_TILE)
                xt = xp.tile([128, G, L], f16)
                m8 = m8p.tile([128, G, L], u8)
                mt = mp.tile([128, G, L], f16)
                tt = tp.tile([128, G, L], f16)

                # DRAM row g*128+p -> tile[p, g, :]
                nc.sync.dma_start(
                    xt[:], x_ext[rows, :].rearrange("(g p) l -> p g l", p=128)
                )
                nc.sync.dma_start(
                    m8[:], m_ext[rows, :].rearrange("(g p) l -> p g l", p=128)
                )

                # Act engine: cast mask u8 -> fp16 (keeps DVE free)
                nc.scalar.copy(mt[:], m8[:])

                # x arrives pre-masked from the host (y0 = mask * x), so
                # xt IS y0; run the 9 steps in place on xt.
                gd = G - G_GP  # DVE handles groups [0, gd), GpSimd [gd, G)
                for s in SHIFTS:
                    # t[i] = y[i-s] * m[i]  for i in [s, L);  y[i] += t[i]
                    nc.vector.tensor_mul(
                        tt[:, :gd, s:], xt[:, :gd, : L - s], mt[:, :gd, s:]
                    )
                    nc.vector.tensor_add(
                        xt[:, :gd, s:], xt[:, :gd, s:], tt[:, :gd, s:]
                    )
                    if G_GP:
                        nc.gpsimd.tensor_mul(
                            tt[:, gd:, s:], xt[:, gd:, : L - s], mt[:, gd:, s:]
                        )
                        nc.gpsimd.tensor_add(
                            xt[:, gd:, s:], xt[:, gd:, s:], tt[:, gd:, s:]
                        )

                nc.sync.dma_start(
                    y_ext[rows, :].rearrange("(g p) l -> p g l", p=128), xt[:]
                )

        if reps == 1:
            body()
        else:
            with tc.For_i(0, reps, 1) as iv:
                body(iv)

    if legalize:
        _legalize_waits(nc)
    return nc


_cached = {}


def kernel(x, mask):
    global _last_results
    from concourse.bass_utils import run_bass_kernel_spmd

    x = np.asarray(x)
    m = np.asarray(mask)
    assert x.shape == (B, L) and m.shape == (B, L)
    # Host pre-masking: y0 = where(mask, x, 0) — saves the on-chip init mul
    x16 = np.where(m, x, np.float32(0.0)).astype(np.float16)
    m8 = m.astype(np.uint8)

    if "flat16" not in _cached:
        _cached["flat16"] = _build_flat16_program()
    nc = _cached["flat16"]

    core_ids = list(range(N_CORES))
    in_maps = [
        {
            "x": x16[i * BP : (i + 1) * BP],
            "m": m8[i * BP : (i + 1) * BP],
        }
        for i in core_ids
    ]

    res = run_bass_kernel_spmd(nc, in_maps, core_ids)
    _last_results = res

    out = np.empty((B, L), dtype=np.float32)
    for i in core_ids:
        out[i * BP : (i + 1) * BP] = res.results[i]["y"].astype(np.float32)
    return out
